# revision 1
# baseline (speedup 1.0000x reference)
"""TRN2 Bass kernel for nn_Decoder (GRU + Bahdanau attention + vocab proj).

8 NeuronCores, data-parallel over batch (4 examples/core), no collectives.
Host: embedding gather, weight repack. Device: phase1 precompute
(x@Wx biases, feat@W1 transposed), phase2 recurrent loop (transposed
weight-stationary fp32r matmuls, 128-partition packed gates, bf16
attention), phase3 batched vocab projection streamed from DRAM.
"""
import sys

sys.path.insert(0, "/opt/trn_rl_repo")

from contextlib import ExitStack

import numpy as np
import concourse.bass as bass
import concourse.mybir as mybir
import concourse.tile as tile
from concourse import bacc
from concourse.bass_utils import run_bass_kernel_spmd
from concourse.masks import make_identity

dt = mybir.dt
AF = mybir.ActivationFunctionType
ALU = mybir.AluOpType
ds = bass.ds

VOCAB = 5000
EMB = 256
RNN = 512
FEAT = 512
L = 400
B = 32
T = 150
ATT = 512
NCORES = 8
BL = B // NCORES  # 4

import os
_H = {"fp16": dt.float16, "bf16": dt.bfloat16, "fp32": dt.float32}[
    os.environ.get("HALF_MODE", "fp16")]
f32, f32r, bf16 = dt.float32, dt.float32r, _H
hf16 = dt.float16  # fixed fp16 pieces (f_sb, wT, transposes)
LPOOL_BUFS = int(os.environ.get("LPOOL_BUFS", "2"))
STG_BUFS = int(os.environ.get("STG_BUFS", "2"))

_CACHE = {}


def _build_nc(t_steps=T, reps=1):
    nc = bacc.Bacc("TRN2", target_bir_lowering=False, debug=False,
                   num_devices=NCORES)
    TS = t_steps

    d_feat = nc.dram_tensor("feat", [BL, L, FEAT], f32, kind="ExternalInput").ap()
    d_xt = nc.dram_tensor("xt", [EMB + 1, TS * BL], f32, kind="ExternalInput").ap()
    d_wxb = nc.dram_tensor("wxb", [EMB + 1, 3 * RNN], f32, kind="ExternalInput").ap()
    d_g1 = nc.dram_tensor("g1", [1024, 1024], f32, kind="ExternalInput").ap()
    d_g2 = nc.dram_tensor("g2", [512, 512], f32, kind="ExternalInput").ap()
    d_g3 = nc.dram_tensor("g3", [513, 512], f32, kind="ExternalInput").ap()
    d_w1 = nc.dram_tensor("w1", [FEAT, ATT], f32, kind="ExternalInput").ap()
    d_w2 = nc.dram_tensor("w2", [RNN, ATT], f32, kind="ExternalInput").ap()
    d_wv = nc.dram_tensor("wv", [ATT, 1], f32, kind="ExternalInput").ap()
    d_wo = nc.dram_tensor("wo", [RNN + FEAT, RNN], f32, kind="ExternalInput").ap()
    d_wp = nc.dram_tensor("wp", [RNN, VOCAB], f32, kind="ExternalInput").ap()
    d_initT = nc.dram_tensor("initT", [RNN, BL], f32, kind="ExternalInput").ap()
    d_out = nc.dram_tensor("out", [TS * BL, VOCAB], f32,
                           kind="ExternalOutput").ap()

    with tile.TileContext(nc) as tc, ExitStack() as ctx:
        wpool = ctx.enter_context(tc.tile_pool(name="wpool", bufs=1))
        stg = ctx.enter_context(tc.tile_pool(name="stg", bufs=STG_BUFS))
        lpool = ctx.enter_context(tc.tile_pool(name="lpool", bufs=LPOOL_BUFS))
        ppool = ctx.enter_context(tc.tile_pool(name="ppool", bufs=1, space="PSUM"))
        p1pool = ctx.enter_context(tc.tile_pool(name="p1pool", bufs=1, space="PSUM"))

        # ---------- weight staging (fp32 DMA -> convert), 2048-col chunks ----
        def load_conv(dram_rows, total_cols, dst_tile, row0=0, pk=128,
                      dst_off=0):
            """DMA fp32 rows [row0:row0+pk] x total_cols into dst (converted),
            chunking columns by 2048."""
            for c0 in range(0, total_cols, 2048):
                cw = min(2048, total_cols - c0)
                st = stg.tile([128, 2048], f32, tag="stage")
                nc.sync.dma_start(st[:pk, :cw], dram_rows[row0:row0 + pk,
                                                          c0:c0 + cw])
                nc.vector.tensor_copy(
                    dst_tile[:pk, dst_off + c0:dst_off + c0 + cw],
                    st[:pk, :cw])

        def wchunks(dram, K, M, dtype, name):
            """SBUF lhsT-chunk layout [128, (kc*mc)*128]; slice (kc, mc) at
            cols (kc*mc_n + mc)*128."""
            kc_n, mc_n = K // 128, M // 128
            tl = wpool.tile([128, kc_n * mc_n * 128], dtype, tag=name)
            for k in range(kc_n):
                load_conv(dram, M, tl, row0=k * 128, dst_off=k * mc_n * 128)
            return tl

        g1_t = wchunks(d_g1, 1024, 1024, f32r, "g1")
        g2_t = wchunks(d_g2, 512, 512, f32r, "g2")
        g3_t = wchunks(d_g3, 512, 512, f32r, "g3")
        g3b_t = wpool.tile([1, 512], f32r, tag="g3b")
        st3b = stg.tile([1, 512], f32, tag="smallstage")
        nc.sync.dma_start(st3b[:], d_g3[512:513, :])
        nc.vector.tensor_copy(g3b_t[:], st3b[:])
        w2_t = wchunks(d_w2, RNN, ATT, f32r, "w2")
        wo_t = wchunks(d_wo, RNN + FEAT, RNN, f32r, "wo")
        w1_t = wchunks(d_w1, FEAT, ATT, bf16, "w1")

        # Vblk [128, (kc,b), BL] bf16: col b of slice (kc,b) = V chunk kc
        vblk = wpool.tile([128, 4 * BL, BL], bf16)
        nc.vector.memset(vblk[:], 0.0)
        vst = stg.tile([128, 4, 1], f32, tag="smallstage")
        nc.sync.dma_start(vst[:], d_wv.rearrange("(kc p) o -> p kc o", p=128))
        vbf = wpool.tile([128, 4], bf16)
        nc.vector.tensor_copy(vbf[:].rearrange("p (k o) -> p k o", o=1), vst[:])
        for k in range(4):
            for b in range(BL):
                nc.vector.tensor_copy(vblk[:, k * BL + b, b:b + 1],
                                      vbf[:, k:k + 1])

        # xt [257 rows] -> xt_t [128, 3, TS*BL] fp32r (chunk2 = bias row)
        xt_t = wpool.tile([128, 3 * TS * BL], f32r, tag="bigshare")
        zst = stg.tile([128, 2048], f32, tag="stage")
        nc.vector.memset(zst[:], 0.0)
        nc.vector.tensor_copy(xt_t[:, 2 * TS * BL:3 * TS * BL],
                              zst[:, :TS * BL])
        for k in range(2):
            load_conv(d_xt, TS * BL, xt_t, row0=k * 128, dst_off=k * TS * BL)
        load_conv(d_xt, TS * BL, xt_t, row0=256, pk=1, dst_off=2 * TS * BL)

        # wxb [257, 1536] -> [128, 3, 1536] fp32r
        wxb_t = wpool.tile([128, 3 * 1536], f32r, tag="wxb")
        nc.vector.tensor_copy(wxb_t[:, 2 * 1536:3 * 1536], zst[:, :1536])
        for k in range(2):
            load_conv(d_wxb, 1536, wxb_t, row0=k * 128, dst_off=k * 1536)
        load_conv(d_wxb, 1536, wxb_t, row0=256, pk=1, dst_off=2 * 1536)

        # f_sb bf16 [128, (b, lc), FEAT]
        f_sb = wpool.tile([128, BL * 4, FEAT], hf16)
        for b in range(BL):
            for lc in range(4):
                lw = 128 if lc < 3 else L - 384
                fst = stg.tile([128, FEAT], f32, tag="fstage")
                nc.sync.dma_start(fst[:lw, :],
                                  d_feat[b, lc * 128:lc * 128 + lw, :])
                nc.vector.tensor_copy(f_sb[:lw, b * 4 + lc, :], fst[:lw, :])

        id32 = wpool.tile([128, 128], f32)
        make_identity(nc, id32[:])
        idbf = wpool.tile([128, 128], hf16)
        nc.vector.tensor_copy(idbf[:], id32[:])

        # recurrent state
        acts = wpool.tile([128, 36], f32r, tag="acts")   # lo(16) st(16) one(4)
        stateF = wpool.tile([128, 16], f32r, tag="stateF")
        zsm = stg.tile([128, 36], f32, tag="smallstage")
        nc.vector.memset(zsm[:], 0.0)
        nc.vector.tensor_copy(acts[:], zsm[:])
        osm = stg.tile([1, 4], f32, tag="smallstage")
        nc.vector.memset(osm[:], 1.0)
        nc.vector.tensor_copy(acts[:1, 32:36], osm[:])
        ist = stg.tile([128, 4, 4], f32, tag="smallstage")
        nc.sync.dma_start(ist[:], d_initT.rearrange("(kc p) b -> p kc b", p=128))
        nc.vector.tensor_copy(acts[:, 16:32].rearrange("p (k b) -> p k b", k=4),
                              ist[:])
        nc.vector.tensor_copy(stateF[:].rearrange("p (k b) -> p k b", k=4),
                              ist[:])

        # ---------------- phase 1: mxAT [128, TS, 48] bf16 ----------------
        mxAT = wpool.tile([128, TS * 12 * BL], bf16, tag="mxAT")
        for g in range(12):
            for n0 in range(0, TS * BL, 512):
                n1 = min(TS * BL, n0 + 512)
                pmx = p1pool.tile([128, 512], f32, tag="p1")
                for kcc in range(3):
                    nc.tensor.matmul(
                        pmx[:, :n1 - n0],
                        wxb_t[:, kcc * 1536 + g * 128:kcc * 1536 + (g + 1) * 128],
                        xt_t[:, kcc * TS * BL + n0:kcc * TS * BL + n1],
                        start=(kcc == 0), stop=(kcc == 2))
                t0, tn = n0 // BL, (n1 - n0) // BL
                # scatter (t,b)-cols into [t, g*BL+b]
                dst = mxAT[:].rearrange("p (t g) -> p t g", g=12 * BL)[
                    :, t0:t0 + tn, g * BL:(g + 1) * BL]
                src = pmx[:, :n1 - n0].rearrange("p (t b) -> p t b", b=BL)
                if g % 2 == 0:
                    nc.vector.tensor_copy(dst, src)
                else:
                    nc.scalar.copy(dst, src)

        # ---------------- phase 1b: fT, fpT ----------------
        big = wpool.tile([128, 4 * BL * L], bf16, tag="bigshare")  # fT
        for b in range(BL):
            for lc in range(4):
                lw = 128 if lc < 3 else L - 384
                for vc in range(4):
                    ptr = p1pool.tile([128, 128], hf16, tag="p1t")
                    nc.tensor.transpose(
                        ptr[:, :lw],
                        f_sb[:lw, b * 4 + lc, vc * 128:(vc + 1) * 128],
                        idbf[:lw, :lw])
                    dst = big[:].rearrange("p (vc n) -> p vc n", vc=4)[
                        :, vc, b * L + lc * 128:b * L + lc * 128 + lw]
                    if (b + lc + vc) % 2 == 0:
                        nc.vector.tensor_copy(dst, ptr[:, :lw])
                    else:
                        nc.scalar.copy(dst, ptr[:, :lw])
        fpT = wpool.tile([128, 4 * BL * L], bf16, tag="wxb")
        for ac in range(4):
            for b in range(BL):
                pfp = p1pool.tile([128, 512], f32, tag="p1")
                for kc in range(4):
                    nc.tensor.matmul(
                        pfp[:, :L],
                        w1_t[:, (kc * 4 + ac) * 128:(kc * 4 + ac + 1) * 128],
                        big[:].rearrange("p (vc n) -> p vc n", vc=4)[
                            :, kc, b * L:(b + 1) * L],
                        start=(kc == 0), stop=(kc == 3))
                dst = fpT[:].rearrange("p (ac n) -> p ac n", ac=4)[
                    :, ac, b * L:(b + 1) * L]
                if (ac + b) % 2 == 0:
                    nc.vector.tensor_copy(dst, pfp[:, :L])
                else:
                    nc.scalar.copy(dst, pfp[:, :L])

        # outsT shares the fT slot (fT dead after fpT) [128, TS, 16] f32r
        outsT = wpool.tile([128, 4 * TS * BL], f32r, tag="bigshare")

        # ---------------- phase 2: recurrent loop ----------------
        def step(o48, o4):
            pgh = ppool.tile([128, 12 * BL], f32, tag="pgh")
            for f in hdep_thunks(pgh):
                f()
            pg = ppool.tile([128, 12 * BL], f32, tag="pg")
            emit_odep(pg, acts[:, 0:16])
            step_tail(pg, pgh, o48)
            step_attn(o4, [], write_acts=True)

        def step_tail(pg, pgh, o48):
            mxs = mxAT[:, ds(o48, 12 * BL)]
            # split z,r from xh so the tanh chain starts before g2 finishes
            gs = lpool.tile([128, 8 * BL], f32, tag="gs")
            nc.vector.tensor_tensor(gs[:], pg[:, :8 * BL], mxs[:, :8 * BL],
                                    ALU.add)
            zra = lpool.tile([128, 8 * BL], f32, tag="zra")
            nc.vector.tensor_tensor(zra[:], gs[:], pgh[:, :8 * BL], ALU.add)
            tau = lpool.tile([128, 8 * BL], f32, tag="tau")
            nc.scalar.activation(tau[:], zra[:], AF.Tanh, scale=0.5)
            gsx = lpool.tile([128, 4 * BL], f32, tag="gsx")
            nc.vector.tensor_tensor(gsx[:], pg[:, 8 * BL:12 * BL],
                                    mxs[:, 8 * BL:12 * BL], ALU.add)
            zr = lpool.tile([128, 8 * BL], f32, tag="zr")
            nc.vector.tensor_scalar(zr[:], tau[:], 0.5, 0.5, ALU.mult, ALU.add)
            rrh = lpool.tile([128, 4 * BL], f32, tag="rrh")
            nc.vector.tensor_tensor(rrh[:], zr[:, 4 * BL:8 * BL],
                                    pgh[:, 8 * BL:12 * BL], ALU.mult)
            hha = lpool.tile([128, 4 * BL], f32, tag="hha")
            nc.vector.tensor_tensor(hha[:], gsx[:], rrh[:], ALU.add)
            hh = lpool.tile([128, 4 * BL], f32, tag="hh")
            nc.scalar.activation(hh[:], hha[:], AF.Tanh)
            dd = lpool.tile([128, 4 * BL], f32, tag="dd")
            nc.vector.tensor_tensor(dd[:], stateF[:], hh[:], ALU.subtract)
            zd = lpool.tile([128, 4 * BL], f32, tag="zd")
            nc.vector.tensor_tensor(zd[:], zr[:, :4 * BL], dd[:], ALU.mult)
            nc.vector.tensor_tensor(stateF[:], hh[:], zd[:], ALU.add)

        _simsafe = os.environ.get("SIM_SAFE", "0") == "1"

        def step_attn(o4, fillers, write_acts=False):
            pmisc = ppool.tile([128, 64], f32, tag="pmisc")
            po = ppool.tile([128, 16], f32, tag="po")
            fillers = fillers + wo1_thunks(po)
            if _simsafe:
                nc.vector.memset(pmisc[:, 16:32], 0.0)
            for m in range(4):      # aT = (h @ W2)^T; ta reads PSUM direct
                for kc in range(4):
                    nc.tensor.matmul(
                        pmisc[:, m * BL:(m + 1) * BL],
                        w2_t[:, (kc * 4 + m) * 128:(kc * 4 + m + 1) * 128],
                        stateF[:, kc * 4:kc * 4 + 4],
                        start=(kc == 0), stop=(kc == 3))
            ps = ppool.tile([BL, L], f32, tag="ps")
            nf = (len(fillers) + 4) // 5 if fillers else 0
            for vc in range(4):
                ta = lpool.tile([128, BL, L], bf16, tag="ta")
                nc.vector.tensor_tensor(
                    ta[:],
                    fpT[:].rearrange("p (vc b l) -> p vc b l", vc=4, b=BL)[
                        :, vc],
                    pmisc[:, vc * BL:(vc + 1) * BL][:, :, None].broadcast_to(
                        [128, BL, L]),
                    ALU.add)
                th = lpool.tile([128, BL, L], bf16, tag="th")
                nc.scalar.activation(th[:], ta[:], AF.Tanh)
                for b in range(BL):
                    nc.tensor.matmul(
                        ps[:], vblk[:, vc * BL + b, :],
                        th[:, b, :],
                        start=(vc == 0 and b == 0),
                        stop=(vc == 3 and b == BL - 1))
                for f in fillers[vc * nf:(vc + 1) * nf]:
                    f()
            for f in fillers[4 * nf:]:  # remainder runs in softmax window
                f()
            w_sb = lpool.tile([BL, L], f32, tag="wsb")
            den = lpool.tile([BL, 1], f32, tag="den")
            nc.scalar.activation(w_sb[:], ps[:], AF.Exp, accum_out=den[:])
            rden = lpool.tile([BL, 1], f32, tag="rden")
            nc.vector.reciprocal(rden[:], den[:])
            wn = lpool.tile([BL, L], f32, tag="wn")
            nc.vector.tensor_scalar(wn[:], w_sb[:], rden[:], None, ALU.mult)
            poS = lpool.tile([128, 4 * BL], f32, tag="poS")
            nc.vector.tensor_copy(poS[:], po[:])
            for lc in range(4):
                lw = 128 if lc < 3 else L - 384
                nc.tensor.transpose(
                    pmisc[:lw, 16 + lc * BL:16 + (lc + 1) * BL],
                    wn[:, lc * 128:lc * 128 + lw], id32[:BL, :BL])
            wT = lpool.tile([128, 4 * BL], hf16, tag="wT")
            nc.vector.tensor_copy(wT[:], pmisc[:, 16:32])
            for b in range(BL):     # ctx^T
                for vc in range(4):
                    for lc in range(4):
                        lw = 128 if lc < 3 else L - 384
                        nc.tensor.matmul(
                            pmisc[:, 32 + vc * BL + b:32 + vc * BL + b + 1],
                            f_sb[:lw, b * 4 + lc, vc * 128:(vc + 1) * 128],
                            wT[:lw, lc * BL + b:lc * BL + b + 1],
                            start=(lc == 0), stop=(lc == 3))
            ctxT = lpool.tile([128, 4 * BL], f32r, tag="ctxT")
            nc.vector.tensor_copy(ctxT[:], pmisc[:, 32:48])
            for m in range(4):      # ctx @ Wo2, transposed out
                for kc in range(4):
                    nc.tensor.matmul(
                        pmisc[:, 48 + m * BL:48 + (m + 1) * BL],
                        wo_t[:, ((4 + kc) * 4 + m) * 128:((4 + kc) * 4 + m + 1) * 128],
                        ctxT[:, kc * BL:(kc + 1) * BL],
                        start=(kc == 0), stop=(kc == 3))
            osp = lpool.tile([128, 4 * BL], f32, tag="osp")
            nc.vector.tensor_tensor(osp[:], pmisc[:, 48:64], poS[:], ALU.add)
            osF = lpool.tile([128, 4 * BL], f32r, tag="osF")
            nc.scalar.activation(osF[:], osp[:], AF.Tanh)
            if write_acts:
                nc.vector.tensor_copy(acts[:, 0:16], osF[:])
            nc.vector.tensor_copy(
                outsT[:].rearrange("p (v n) -> p v n", v=4)[:, :, ds(o4, BL)],
                osF[:].rearrange("p (v b) -> p v b", v=4))
            return osF

        # --- software-pipelined emission (unrolled path) ---------------
        # Gate matmuls split: h-dependent half (g1 state part, g3, bias)
        # only needs h_t, ready mid-step; o-dependent half (g1 out part,
        # g2) needs o_t (step end). Next step's h-half is emitted inside
        # this step's attention window so the in-order PE fills its ACT
        # stalls with it. pg uses a single has_written-clearing start=True
        # on the first h-half matmul; later matmuls rely on per-element
        # overwrite-then-accumulate.
        def hdep_thunks(pgh):
            items = [("g1", m, kc) for m in range(8) for kc in range(4, 8)]
            items += [("g3", m, kc) for m in range(4) for kc in range(5)]

            def mk(item):
                which, m, kc = item

                def go():
                    if which == "g1":
                        nc.tensor.matmul(
                            pgh[:, m * BL:(m + 1) * BL],
                            g1_t[:, (kc * 8 + m) * 128:(kc * 8 + m + 1) * 128],
                            stateF[:, (kc - 4) * 4:(kc - 4) * 4 + 4],
                            start=(kc == 4), stop=(kc == 7))
                    elif kc < 4:
                        nc.tensor.matmul(
                            pgh[:, (8 + m) * BL:(9 + m) * BL],
                            g3_t[:, (kc * 4 + m) * 128:(kc * 4 + m + 1) * 128],
                            stateF[:, kc * 4:kc * 4 + 4],
                            start=(kc == 0), stop=False)
                    else:
                        nc.tensor.matmul(
                            pgh[:, (8 + m) * BL:(9 + m) * BL],
                            g3b_t[:, m * 128:(m + 1) * 128],
                            acts[:1, 32:36], start=False, stop=True)
                return go
            return [mk(i) for i in items]

        def wo1_thunks(po):
            def mk(m, kc):
                def go():
                    nc.tensor.matmul(
                        po[:, m * BL:(m + 1) * BL],
                        wo_t[:, (kc * 4 + m) * 128:(kc * 4 + m + 1) * 128],
                        stateF[:, kc * 4:kc * 4 + 4],
                        start=(kc == 0), stop=(kc == 3))
                return go
            return [mk(m, kc) for m in range(4) for kc in range(4)]

        def emit_odep(pg, o_src):
            for m in range(8):      # z, r: last-output half (kc 0-3)
                for kc in range(4):
                    nc.tensor.matmul(
                        pg[:, m * BL:(m + 1) * BL],
                        g1_t[:, (kc * 8 + m) * 128:(kc * 8 + m + 1) * 128],
                        o_src[:, kc * 4:kc * 4 + 4],
                        start=(kc == 0), stop=(kc == 3))
            for m in range(4):      # xh
                for kc in range(4):
                    nc.tensor.matmul(
                        pg[:, (8 + m) * BL:(9 + m) * BL],
                        g2_t[:, (kc * 4 + m) * 128:(kc * 4 + m + 1) * 128],
                        o_src[:, kc * 4:kc * 4 + 4],
                        start=(kc == 0), stop=(kc == 3))

        if os.environ.get("UNROLL", "1") == "1":
            for _ in range(reps):
                pgh_next = ppool.tile([128, 12 * BL], f32, tag="pgh")
                for f in hdep_thunks(pgh_next):
                    f()
                o_prev = acts[:, 0:16]
                for t in range(TS):
                    pgh_cur = pgh_next
                    pg = ppool.tile([128, 12 * BL], f32, tag="pg")
                    emit_odep(pg, o_prev)
                    step_tail(pg, pgh_cur, t * 12 * BL)
                    if t + 1 < TS:
                        pgh_next = ppool.tile([128, 12 * BL], f32, tag="pgh")
                        fillers = hdep_thunks(pgh_next)
                    else:
                        fillers = []
                    o_prev = step_attn(t * BL, fillers)
        else:
            for _ in range(reps):
                with tc.For_i(0, TS, 1) as tix:
                    step(tix * (12 * BL), tix * BL)

        # ---------------- phase 3: logits ----------------
        MT = (TS * BL + 127) // 128
        for vs in range((VOCAB + 511) // 512):
            v0 = vs * 512
            vw = min(512, VOCAB - v0)
            wst = stg.tile([128, 4, 512], f32, tag="stage")
            nc.sync.dma_start(
                wst[:, :, :vw],
                d_wp[:, v0:v0 + vw].rearrange("(kc p) v -> p kc v", p=128))
            wpr = wpool.tile([128, 4, 512], f32r, tag="wxb")  # reuse slot
            if vs % 2 == 0:
                nc.vector.tensor_copy(wpr[:, :, :vw], wst[:, :, :vw])
            else:
                nc.scalar.copy(wpr[:, :, :vw], wst[:, :, :vw])
            for mt in range(MT):
                m0 = mt * 128
                mw = min(128, TS * BL - m0)
                plg = p1pool.tile([128, 512], f32, tag="p1")
                for kc in range(4):
                    lhs = outsT[:].rearrange("p (v n) -> p v n", v=4)[
                        :, kc, m0:m0 + mw]
                    nc.tensor.matmul(plg[:mw, :vw], lhs, wpr[:, kc, :vw],
                                     start=(kc == 0), stop=(kc == 3))
                lsb = stg.tile([128, 512], f32, tag="lsb")
                if mt % 2 == 0:
                    nc.vector.tensor_copy(lsb[:mw, :vw], plg[:mw, :vw])
                else:
                    nc.scalar.copy(lsb[:mw, :vw], plg[:mw, :vw])
                nc.sync.dma_start(d_out[m0:m0 + mw, v0:v0 + vw],
                                  lsb[:mw, :vw])

    nc.compile()
    return nc


def _prep_inputs(features, init_state, emb, gru_kernel, gru_rec_kernel,
                 gru_bias, attn_W1, attn_W2, attn_V, out_W, proj_W, formula,
                 t_steps=T):
    f = np.float32
    features = np.asarray(features, f)
    init_state = np.asarray(init_state, f)
    emb = np.asarray(emb, f)
    gru_kernel = np.asarray(gru_kernel, f)
    gru_rec_kernel = np.asarray(gru_rec_kernel, f)
    gru_bias = np.asarray(gru_bias, f)
    attn_W1 = np.asarray(attn_W1, f)
    attn_W2 = np.asarray(attn_W2, f)
    attn_V = np.asarray(attn_V, f)
    out_W = np.asarray(out_W, f)
    proj_W = np.asarray(proj_W, f)
    formula = np.asarray(formula)

    Wx, Wl = gru_kernel[:EMB], gru_kernel[EMB:]
    Wr = gru_rec_kernel
    b0, b1 = gru_bias[0], gru_bias[1]
    brow = b0.copy()
    brow[:1024] += b1[:1024]
    wxb = np.concatenate([Wx, brow[None, :]], 0)
    g1 = np.concatenate([Wl[:, :1024], Wr[:, :1024]], 0)
    g2 = np.ascontiguousarray(Wl[:, 1024:])
    g3 = np.concatenate([Wr[:, 1024:], b1[None, 1024:]], 0)
    x_all = emb[formula[:, :t_steps]]  # [B, TS, EMB]

    in_maps = []
    for c in range(NCORES):
        bsl = slice(c * BL, (c + 1) * BL)
        xt = np.ascontiguousarray(
            x_all[bsl].transpose(2, 1, 0).reshape(EMB, t_steps * BL))
        xt = np.concatenate([xt, np.ones((1, t_steps * BL), f)], 0)
        in_maps.append({
            "feat": np.ascontiguousarray(features[bsl]),
            "xt": xt, "wxb": wxb, "g1": g1, "g2": g2, "g3": g3,
            "w1": attn_W1, "w2": attn_W2, "wv": attn_V.reshape(ATT, 1),
            "wo": out_W, "wp": proj_W,
            "initT": np.ascontiguousarray(init_state[bsl].T),
        })
    return in_maps


def _get_nc(t_steps=T, reps=1):
    key = (t_steps, reps)
    if key not in _CACHE:
        _CACHE[key] = _build_nc(t_steps=t_steps, reps=reps)
    return _CACHE[key]


def run(inputs, t_steps=T, reps=1):
    nc = _get_nc(t_steps, reps)
    in_maps = _prep_inputs(**inputs, t_steps=t_steps)
    res = run_bass_kernel_spmd(nc, in_maps, core_ids=list(range(NCORES)))
    outs = []
    for c in range(NCORES):
        lg = res.results[c]["out"]
        outs.append(lg.reshape(t_steps, BL, VOCAB).transpose(1, 0, 2))
    return np.concatenate(outs, 0).astype(np.float32)


def kernel(**inputs):
    return run(inputs, t_steps=T, reps=1)


